# revision 1
# baseline (speedup 1.0000x reference)
"""Block-circulant linear layer on 8 Trainium2 NeuronCores.

Math: y[n, (j,b)] = sum_i circconv(x[n,i,:], c[j,i,:])[b] + bias.
Factorized via packed-real-FFT (halfcomplex, 128 slots of (re,im)):

  stage A (rfft):    t1 = F_pack^T @ x^T     per in-block i, block-major out
  permute A->B:      slot-major regroup (DMA row shuffle)
  stage B (mixing):  t2 = W2_g^T @ t1p       block-diagonal per slot-group g
  permute B->C:      block-major regroup (DMA row shuffle)
  stage C (irfft):   y[tok, b] = t2p_tile^T @ G   (token-major output) + bias

All matmuls run as fp32r (full-rate fp32) with N=256 moving columns.
Sharding: data-parallel, 1024 tokens per core; weights replicated.
Host preprocessing: transpose+chunk x shards, build F/W2/G/bias layouts.
"""

import numpy as np

try:
    import ml_dtypes
    _BF16 = ml_dtypes.bfloat16
except ImportError:  # pragma: no cover
    _BF16 = None

MID_BF16 = True

BLOCK = 256
NB = 16          # in/out blocks
NSLOT = 128      # frequency slots (halfcomplex pairs)
N_CORES = 8
TOK_PER_CORE = 1024
CHUNK = 256      # tokens per pipeline chunk
N_CHUNKS = TOK_PER_CORE // CHUNK
IN_F = NB * BLOCK  # 4096


def _build_weights(c: np.ndarray):
    """Host-side weight construction (float64 for accuracy, cast to f32)."""
    B, K = BLOCK, NSLOT
    b = np.arange(B)
    k = np.arange(K)
    theta = 2 * np.pi * np.outer(b, k) / B
    F_re = np.cos(theta)
    F_im = -np.sin(theta)
    F_im[:, 0] = (-1.0) ** b            # Nyquist column in the c=1 half, k=0
    F_pack = np.concatenate([F_re, F_im], axis=1)   # [256 b, 256 (c,k)]

    G_re = np.zeros((K, B))
    G_im = np.zeros((K, B))
    kk = np.arange(1, K)
    th = 2 * np.pi * np.outer(kk, b) / B
    G_re[1:] = 2.0 * np.cos(th) / B
    G_re[0] = 1.0 / B
    G_im[1:] = -2.0 * np.sin(th) / B
    G_im[0] = ((-1.0) ** b) / B
    G_pack = np.stack([G_re, G_im], axis=0)          # [2, 128 k, 256 b]

    Cf = np.fft.rfft(c.astype(np.float64), axis=-1)  # [j, i, 129]
    A = Cf.real
    Bm = Cf.imag
    W2 = np.zeros((32, 128, 128))
    for g in range(32):
        for s in range(4):
            ks = 4 * g + s
            blk = np.zeros((32, 32))                 # rows (c,i) -> cols (c',j)
            if ks == 0:
                blk[0:16, 0:16] = A[:, :, 0].T
                blk[16:32, 16:32] = A[:, :, 128].T
            else:
                a = A[:, :, ks].T
                bb = Bm[:, :, ks].T
                blk[0:16, 0:16] = a
                blk[16:32, 0:16] = -bb
                blk[0:16, 16:32] = bb
                blk[16:32, 16:32] = a
            W2[g, 32 * s:32 * s + 32, 32 * s:32 * s + 32] = blk

    f_host = (
        F_pack.reshape(2, 128, 2, 128).transpose(1, 0, 2, 3).reshape(128, 512)
    )  # [p=b_local, bh*256 + ch*128 + k]
    w2_host = W2.transpose(1, 0, 2).reshape(128, 32 * 128)   # [p, 128g + m]
    g_host = G_pack.transpose(1, 0, 2).reshape(128, 512)     # [k, ch*256 + b]
    return (
        f_host.astype(np.float32),
        w2_host.astype(np.float32),
        g_host.astype(np.float32),
    )


_NC_CACHE = {}
_ONES = np.ones((1, 128), dtype=np.float32)


def _build_module(skip_permutes=False, repeat=1, perm_mode="3way", mid_bf16=True,
                  io_on_scalar=True, y_bf16=True, psum_bufs=(2, 3, 3), mid_bufs=6):
    """Build + compile the per-core Bass module (cached)."""
    key = ("nc", skip_permutes, repeat, perm_mode, mid_bf16, io_on_scalar, y_bf16,
           psum_bufs, mid_bufs)
    if key in _NC_CACHE:
        return _NC_CACHE[key]

    import concourse.bass as bass  # noqa: F401
    import concourse.mybir as mybir
    import concourse.tile as tile
    from concourse import bacc

    f32 = mybir.dt.float32
    f32r = mybir.dt.float32r
    bf16 = mybir.dt.bfloat16
    mid_dt = bf16 if mid_bf16 else f32r
    ps_dt = bf16 if mid_bf16 else f32

    nc = bacc.Bacc("TRN2", target_bir_lowering=False, debug=False)

    xt_d = nc.dram_tensor(
        "xt", [N_CHUNKS, 128, 32, CHUNK], mid_dt, kind="ExternalInput"
    )
    f_d = nc.dram_tensor("fw", [128, 512], mid_dt, kind="ExternalInput")
    w2_d = nc.dram_tensor("w2", [128, 4096], mid_dt, kind="ExternalInput")
    g_d = nc.dram_tensor("gw", [128, 512], mid_dt, kind="ExternalInput")
    bias_d = nc.dram_tensor("biasr", [1, IN_F], mid_dt, kind="ExternalInput")
    ones_d = nc.dram_tensor("ones", [1, 128], mid_dt, kind="ExternalInput")
    y_dt = (bf16 if mid_bf16 else f32) if y_bf16 else f32
    y_d = nc.dram_tensor("y", [TOK_PER_CORE, IN_F], y_dt, kind="ExternalOutput")

    with tile.TileContext(nc) as tc:
        with (
            tc.tile_pool(name="wpool", bufs=1) as wpool,
            tc.tile_pool(name="pin", bufs=2) as pin,
            tc.tile_pool(name="mid", bufs=mid_bufs) as mid,
            tc.tile_pool(name="psA", bufs=psum_bufs[0], space="PSUM") as psA,
            tc.tile_pool(name="psB", bufs=psum_bufs[1], space="PSUM") as psB,
            tc.tile_pool(name="psC", bufs=psum_bufs[2], space="PSUM") as psC,
        ):
            f_sb = wpool.tile([128, 512], mid_dt, tag="fw")
            w2_sb = wpool.tile([128, 4096], mid_dt, tag="w2")
            g_sb = wpool.tile([128, 512], mid_dt, tag="gw")
            bias_sb = wpool.tile([1, IN_F], mid_dt, tag="bias")
            ones_sb = wpool.tile([1, 128], mid_dt, tag="ones")
            nc.sync.dma_start(out=f_sb[:], in_=f_d[:])
            nc.sync.dma_start(out=w2_sb[:], in_=w2_d[:])
            nc.sync.dma_start(out=g_sb[:], in_=g_d[:])
            nc.sync.dma_start(out=bias_sb[:], in_=bias_d[:])
            nc.sync.dma_start(out=ones_sb[:], in_=ones_d[:])


            def perm_eng(g):
                if perm_mode == "sync":
                    return nc.sync
                if perm_mode == "split2":
                    return nc.gpsimd if g % 2 else nc.sync
                if perm_mode == "scalar_sync":
                    return nc.scalar if g % 2 else nc.sync
                if perm_mode == "3way":
                    return (nc.sync, nc.scalar, nc.gpsimd)[g % 3]
                raise ValueError(perm_mode)

            evac_n = [0]

            def evac(dst, srcp):
                if evac_n[0] % 2 == 0:
                    nc.vector.tensor_copy(dst, srcp)
                else:
                    nc.scalar.copy(dst, srcp)
                evac_n[0] += 1

            for ci_rep in range(N_CHUNKS * repeat):
                ci = ci_rep % N_CHUNKS
                # ---- load x^T chunk: [128 p, 32 f, 256 t] ----
                xts = pin.tile([128, 8192], mid_dt, tag="pin")
                io_eng = nc.scalar if io_on_scalar else nc.sync
                io_eng.dma_start(
                    out=xts[:].rearrange("p (f t) -> p f t", f=32),
                    in_=xt_d[ci],
                )

                # ---- stage A: rfft per in-block ----
                # t1 chunk q1 = 16*ch + i; quad psum bank (bf16) per (ch, i0..i0+3)
                t1 = mid.tile([128, 8192], mid_dt, tag="mid")
                for ch in range(2):
                    for i0 in range(0, NB, 2):
                        ps = psA.tile([128, 512], f32, tag="psA")
                        for ii in range(i0, i0 + 2):
                            off = (ii - i0) * 256
                            for bh in range(2):
                                nc.tensor.matmul(
                                    ps[:, off: off + 256],
                                    f_sb[:, bh * 256 + ch * 128: bh * 256 + ch * 128 + 128],
                                    xts[:, (2 * ii + bh) * 256: (2 * ii + bh) * 256 + 256],
                                    start=(bh == 0),
                                    stop=(bh == 1),
                                )
                        q1 = 16 * ch + i0
                        evac(t1[:, q1 * 256: q1 * 256 + 512], ps[:])

                # ---- permute A->B: slot-major regroup ----
                # t1p[32s+16c+i, g, t] = t1[k=4g+s, 16c+i, t]
                if skip_permutes:
                    t1p = t1
                else:
                    t1p = mid.tile([128, 8192], mid_dt, tag="mid")
                    t1v = t1[:].rearrange("p (q m) -> p q m", m=CHUNK)
                    for g in range(32):
                        perm_eng(g).dma_start(
                            out=t1p[:, g * 256: g * 256 + 256],
                            in_=t1v[4 * g: 4 * g + 4],
                        )

                # ---- stage B: per-slot complex mixing (block-diagonal) ----
                t2 = mid.tile([128, 8192], mid_dt, tag="mid")
                for g0 in range(0, 32, 2):
                    ps = psB.tile([128, 512], f32, tag="psB")
                    for gg in range(g0, g0 + 2):
                        off = (gg - g0) * 256
                        nc.tensor.matmul(
                            ps[:, off: off + 256],
                            w2_sb[:, gg * 128: gg * 128 + 128],
                            t1p[:, gg * 256: gg * 256 + 256],
                            start=True,
                            stop=True,
                        )
                    evac(t2[:, g0 * 256: g0 * 256 + 512], ps[:])

                # ---- permute B->C: block-major regroup ----
                # t2p[k=4g+s, 16c+j, t] = t2[32s+16c+j, g, t]
                if skip_permutes:
                    t2p = t2
                else:
                    t2p = mid.tile([128, 8192], mid_dt, tag="mid")
                    t2pv = t2p[:].rearrange("p (q m) -> p q m", m=CHUNK)
                    for g in range(32):
                        perm_eng(g).dma_start(
                            out=t2pv[4 * g: 4 * g + 4],
                            in_=t2[:, g * 256: g * 256 + 256],
                        )

                # ---- stage C: irfft + bias, token-major output ----
                ysb = mid.tile([128, 8192], mid_dt, tag="mid")
                for tsub in range(2):
                    for j0 in range(0, NB, 2):
                        ps = psC.tile([128, 512], f32, tag="psC")
                        for jj in range(j0, j0 + 2):
                            off = (jj - j0) * 256
                            for ch in range(2):
                                q4 = 16 * ch + jj
                                nc.tensor.matmul(
                                    ps[:, off: off + 256],
                                    t2p[:, q4 * 256 + 128 * tsub: q4 * 256 + 128 * tsub + 128],
                                    g_sb[:, ch * 256: ch * 256 + 256],
                                    start=(ch == 0),
                                    stop=False,
                                )
                            nc.tensor.matmul(
                                ps[:, off: off + 256],
                                ones_sb[:, :],
                                bias_sb[:, jj * 256: jj * 256 + 256],
                                start=False,
                                stop=True,
                            )
                        evac(
                            ysb[:, tsub * 4096 + j0 * 256: tsub * 4096 + j0 * 256 + 512],
                            ps[:],
                        )

                # ---- store y chunk ----
                y_eng = nc.sync if y_bf16 else nc.gpsimd
                y_eng.dma_start(
                    out=y_d[ci * 256: ci * 256 + 256, :].rearrange(
                        "(s p) o -> p s o", p=128
                    ),
                    in_=ysb[:].rearrange("p (s o) -> p s o", s=2),
                )

    nc.compile()
    _NC_CACHE[key] = nc
    return nc


def kernel(x: np.ndarray, c: np.ndarray, bias: np.ndarray) -> np.ndarray:
    from concourse.bass_utils import run_bass_kernel_spmd

    batch, seq, in_f = x.shape
    n_tok = batch * seq
    xf = np.ascontiguousarray(x.reshape(n_tok, in_f).astype(np.float32))

    f_host, w2_host, g_host = _build_weights(np.asarray(c, dtype=np.float32))
    bias_host = np.asarray(bias, dtype=np.float32).reshape(1, IN_F)
    ones_host = _ONES
    if MID_BF16:
        f_host = f_host.astype(_BF16)
        w2_host = w2_host.astype(_BF16)
        g_host = g_host.astype(_BF16)
        bias_host = bias_host.astype(_BF16)
        ones_host = _ONES.astype(_BF16)

    nc = _build_module(mid_bf16=MID_BF16)

    in_maps = []
    for core in range(N_CORES):
        shard = xf[core * TOK_PER_CORE:(core + 1) * TOK_PER_CORE]  # [1024, 4096]
        # xt[ci, p, f, t] = shard[ci*256 + t, 128*f + p]
        xt = np.ascontiguousarray(
            shard.reshape(N_CHUNKS, CHUNK, 32, 128).transpose(0, 3, 2, 1)
        )
        if MID_BF16:
            xt = xt.astype(_BF16)
        in_maps.append(
            {
                "xt": xt,
                "fw": f_host,
                "w2": w2_host,
                "gw": g_host,
                "biasr": bias_host,
                "ones": ones_host,
            }
        )

    res = run_bass_kernel_spmd(nc, in_maps, core_ids=list(range(N_CORES)))
    y = np.concatenate(
        [np.asarray(r["y"], dtype=np.float32) for r in res.results], axis=0
    )
    return y.reshape(batch, seq, in_f).astype(x.dtype)

